# revision 17
# baseline (speedup 1.0000x reference)
"""Trainium2 Bass kernel for nn_BatchLossFunction_38534446579748.

Loss:  cos = <pt[b,p,:], ot[b,:]> / (max(||pt||,eps) * max(||ot||,eps))
       v   = sigmoid(1 - cos);  gtv = gt.reshape(B,196)/255
       loss = sum(-log(1 - |v - gtv|) * (gtv*GAMMA + 1)) / B

Strategy (pure data parallel over batch, 8 cores x 256 batches):
  - Layout: batch on SBUF partitions, patch index on the free dim. out_text
    then lives naturally as one row per partition, so the per-row dot and
    sum-of-squares reductions are single fused ops with no broadcasts:
      * ScalarE ACTIVATE(Square, accum_out)      -> sumsq per (b,p) row
      * VectorE SCALAR_TENSOR_TENSOR(accum_out)  -> dot per (b,p) row
    A slice of the sumsq work runs on VectorE to balance the two engines.
  - out_text is pre-normalized by max(||ot||,eps) on-chip, so the dot
    directly yields dots/otn and the epilogue shortens.
  - Per core: 2 groups of 128 batches; patch_tokens streams in [128, 14, 768]
    tiles (5.5 MB DMAs, near peak HBM bw); the last chunks taper so the
    post-DMA compute tail is short.
  - Tiny epilogue on [128, 196] tiles; per-partition partial sums DMA'd out,
    final reduction across partitions/cores on host.
"""

import os
import sys

import numpy as np

if "/opt/trn_rl_repo" not in sys.path:
    sys.path.insert(0, "/opt/trn_rl_repo")

from contextlib import ExitStack

import concourse.bacc as bacc
import concourse.tile as tile
from concourse import mybir
from concourse.bass_utils import run_bass_kernel_spmd

N_CORES = 8
B, P, D = 2048, 197, 768
NP = P - 1          # 196 usable patch tokens
BS = B // N_CORES   # 256 batches per core
PART = 128          # SBUF partitions
G = BS // PART      # 2 groups of 128 batches per core
GAMMA = 3.4
EPS = 1e-8

# chunk plan per group: list of (patch_count, n_rows_with_sumsq_on_dve)
_PLAN0 = [(14, 0)] * 13 + [(14, 14)]
_PLAN1 = [(14, 0)] * 12 + [(14, 4)] + [(7, 0), (4, 0), (3, 0)]
PLANS = [_PLAN0, _PLAN1]

F32 = mybir.dt.float32
ALU = mybir.AluOpType
ACTF = mybir.ActivationFunctionType

_CACHE = {}


def _build():
    nc = bacc.Bacc("TRN2", target_bir_lowering=False, debug=False)

    pt = nc.dram_tensor("patch_tokens", [BS, P, D], F32, kind="ExternalInput")
    ot = nc.dram_tensor("out_text", [BS, D], F32, kind="ExternalInput")
    gt = nc.dram_tensor("gt", [BS, 14, 14], F32, kind="ExternalInput")
    out = nc.dram_tensor("loss_parts", [PART, G], F32, kind="ExternalOutput")

    pt_ap = pt.ap()
    ot_ap = ot.ap()
    gt_ap = gt.ap().rearrange("b h w -> b (h w)")
    out_ap = out.ap()

    with ExitStack() as ctx:
        tc = ctx.enter_context(tile.TileContext(nc))
        xs = ctx.enter_context(tc.tile_pool(name="xs", bufs=4))
        persist = ctx.enter_context(tc.tile_pool(name="persist", bufs=1))
        psum = ctx.enter_context(tc.tile_pool(name="psum", bufs=1, space="PSUM"))

        trash_a = psum.tile([PART, D], F32, tag="trash_a")      # ACT main out
        trash_v = persist.tile([PART, D], F32, tag="trash_v")   # DVE main out
        loss = persist.tile([PART, G], F32, tag="loss")

        # ---- prologue: load + normalize out_text, load gt, precompute W ----
        ots, gtts, ws = [], [], []
        for g in range(G):
            b0 = g * PART
            otile = persist.tile([PART, D], F32, tag=f"ot{g}")
            nc.gpsimd.dma_start(out=otile, in_=ot_ap[b0 : b0 + PART, :])
            otsq = persist.tile([PART, 1], F32, tag=f"otsq{g}")
            nc.scalar.activation(
                out=trash_a, in_=otile, func=ACTF.Square, accum_out=otsq
            )
            otn = persist.tile([PART, 1], F32, tag=f"otn{g}")
            nc.scalar.activation(out=otn, in_=otsq, func=ACTF.Sqrt)
            nc.vector.tensor_scalar_max(out=otn, in0=otn, scalar1=EPS)
            inv_otn = persist.tile([PART, 1], F32, tag=f"inv_otn{g}")
            nc.vector.reciprocal(out=inv_otn, in_=otn)
            nc.vector.tensor_scalar_mul(out=otile, in0=otile, scalar1=inv_otn)
            ots.append(otile)

            gtt = persist.tile([PART, NP], F32, tag=f"gtt{g}")
            nc.gpsimd.dma_start(out=gtt, in_=gt_ap[b0 : b0 + PART, :])
            gtts.append(gtt)
            w = persist.tile([PART, NP], F32, tag=f"w{g}")  # -(GAMMA*gtv + 1)
            nc.scalar.activation(
                out=w, in_=gtt, func=ACTF.Copy, scale=-GAMMA / 255.0, bias=-1.0
            )
            ws.append(w)

        # ---- main streaming loops: sumsq on ACT (mostly), dot on DVE ----
        sss, ssvs, dts, segss = [], [], [], []
        for g in range(G):
            b0 = g * PART
            ss = psum.tile([PART, NP], F32, tag=f"ss{g}")
            ssv = persist.tile([PART, NP], F32, tag=f"ssv{g}")
            dt_ = persist.tile([PART, NP], F32, tag=f"dt{g}")
            sss.append(ss)
            ssvs.append(ssv)
            dts.append(dt_)
            segs = []  # contiguous col segments owned by ACT(ss) vs DVE(ssv)

            def _seg(lo, hi, src):
                if lo >= hi:
                    return
                if segs and segs[-1][2] == src:
                    segs[-1][1] = hi
                else:
                    segs.append([lo, hi, src])

            p0 = 0
            for pc, n_dve in PLANS[g]:
                _seg(p0, p0 + n_dve, "v")
                _seg(p0 + n_dve, p0 + pc, "a")
                x = xs.tile([PART, 14, D], F32, tag="x")
                nc.sync.dma_start(
                    out=x[:, :pc, :],
                    in_=pt_ap[b0 : b0 + PART, 1 + p0 : 1 + p0 + pc, :],
                )
                for j in range(pc):
                    p = p0 + j
                    if j < n_dve:
                        nc.vector.scalar_tensor_tensor(
                            out=trash_v,
                            in0=x[:, j, :],
                            scalar=1.0,
                            in1=x[:, j, :],
                            op0=ALU.mult,
                            op1=ALU.mult,
                            accum_out=ssv[:, p : p + 1],
                        )
                    else:
                        nc.scalar.activation(
                            out=trash_a,
                            in_=x[:, j, :],
                            func=ACTF.Square,
                            accum_out=ss[:, p : p + 1],
                        )
                    nc.vector.scalar_tensor_tensor(
                        out=trash_v,
                        in0=x[:, j, :],
                        scalar=1.0,
                        in1=ots[g],
                        op0=ALU.mult,
                        op1=ALU.mult,
                        accum_out=dt_[:, p : p + 1],
                    )
                p0 += pc
            segss.append(segs)

        # ---- epilogue on [128, 196] tiles (batched by ACT table set) ----
        ptns = []
        for g in range(G):  # Sqrt table set
            ptn = persist.tile([PART, NP], F32, tag=f"ptn{g}")
            for s0, s1, src in segss[g]:
                buf = ssvs[g] if src == "v" else sss[g]
                nc.scalar.activation(
                    out=ptn[:, s0:s1], in_=buf[:, s0:s1], func=ACTF.Sqrt
                )
            ptns.append(ptn)

        coss = []
        for g in range(G):  # DVE: clamp, reciprocal, cosine (dt is dots/otn)
            nc.vector.tensor_scalar_max(out=ptns[g], in0=ptns[g], scalar1=EPS)
            rp = persist.tile([PART, NP], F32, tag=f"rp{g}")
            nc.vector.reciprocal(out=rp, in_=ptns[g])
            cos = persist.tile([PART, NP], F32, tag=f"cos{g}")
            nc.vector.tensor_mul(out=cos, in0=dts[g], in1=rp)
            coss.append(cos)

        vs = []
        for g in range(G):  # Sigmoid table set: v = sigmoid(1 - cos)
            v = persist.tile([PART, NP], F32, tag=f"v{g}")
            nc.scalar.activation(
                out=v, in_=coss[g], func=ACTF.Sigmoid, bias=1.0, scale=-1.0
            )
            vs.append(v)

        omds = []
        for g in range(G):  # DVE: omd = gt/255 - v  (== -(v - gtv) = -diff)
            omd = persist.tile([PART, NP], F32, tag=f"omd{g}")
            nc.vector.scalar_tensor_tensor(
                out=omd,
                in0=gtts[g],
                scalar=1.0 / 255.0,
                in1=vs[g],
                op0=ALU.mult,
                op1=ALU.subtract,
            )
            omds.append(omd)

        lns = []
        for g in range(G):  # Ln table set: L = log(1 - diff)
            ln = persist.tile([PART, NP], F32, tag=f"ln{g}")
            nc.scalar.activation(out=ln, in_=omds[g], func=ACTF.Ln, bias=1.0)
            lns.append(ln)

        for g in range(G):  # loss partials: sum_p L*W per partition
            nc.vector.scalar_tensor_tensor(
                out=trash_v[:, :NP],
                in0=lns[g],
                scalar=1.0,
                in1=ws[g],
                op0=ALU.mult,
                op1=ALU.mult,
                accum_out=loss[:, g : g + 1],
            )

        nc.gpsimd.dma_start(out=out_ap, in_=loss)

    nc.compile()
    return nc


def _get_nc():
    if "nc" not in _CACHE:
        _CACHE["nc"] = _build()
    return _CACHE["nc"]


def _run(in_maps, **kwargs):
    return run_bass_kernel_spmd(_get_nc(), in_maps, core_ids=list(range(N_CORES)), **kwargs)


def _make_in_maps(patch_tokens, out_text, gt):
    patch_tokens = np.ascontiguousarray(np.asarray(patch_tokens, dtype=np.float32))
    out_text = np.ascontiguousarray(np.asarray(out_text, dtype=np.float32))
    gt = np.ascontiguousarray(np.asarray(gt, dtype=np.float32))
    in_maps = []
    for c in range(N_CORES):
        sl = slice(c * BS, (c + 1) * BS)
        in_maps.append(
            {
                "patch_tokens": patch_tokens[sl],
                "out_text": out_text[sl],
                "gt": gt[sl],
            }
        )
    return in_maps


def kernel(patch_tokens, out_text, gt):
    res = _run(_make_in_maps(patch_tokens, out_text, gt))
    total = np.float64(0.0)
    for r in res.results:
        total += r["loss_parts"].astype(np.float64).sum()
    return np.float32(total / B)
